# revision 3
# baseline (speedup 1.0000x reference)
"""Routed (sparse) MoE kernel for Trainium2, expert-parallel over 8 NeuronCores.

Problem: Qwen3-MoE sparse block. T=2048 tokens, H=2048 hidden, E=32 experts,
F=768 intermediate, top-K=8, norm_topk_prob=True.

Strategy:
  * Host: router (logits -> softmax -> top-8 -> renormalize), replicated with
    jax-on-CPU to match the reference's numerics bit-for-bit where possible.
  * Host: gather each expert's routed tokens into a fixed-capacity (512) slot,
    pre-transposed to [H, C] and cast to bf16. Expert e -> core e%8, slot e//8.
    Tokens beyond capacity (rare: mean count is 512) are computed on host in
    fp32 — this keeps the device graph shape input-independent.
  * Device (per core): 4 expert slots. For each slot, the whole SwiGLU FFN in
    a transposed dataflow (tokens on the matmul free axis), bf16 matmuls with
    fp32 PSUM accumulation, silu on ACT, multiply on DVE:
        gT[F,C] = Wg^T x      (lhsT = Wg[H,F] tiles, rhs = xT[H,C] tiles)
        uT[F,C] = Wu^T x
        hT      = silu(gT) * uT
        yT[H,C] = Wd^T h      (lhsT = Wd[F,H] tiles, rhs = hT tiles)
    No on-chip transposes anywhere.
  * Host: combine — out[t] = sum_k w[t,k] * y_{e_k}[t], a per-expert weighted
    scatter-add with unique indices (fp32).
"""

import numpy as np
import ml_dtypes

import concourse.bass as bass  # noqa: F401  (registers engines)
import concourse.mybir as mybir
import concourse.tile as tile
from concourse import bacc
from concourse.bass_utils import run_bass_kernel_spmd

# Model dims (hardcoded per problem spec)
T, H, E, F, K = 2048, 2048, 32, 768, 8
NCORES = 8
SLOTS = E // NCORES  # 4 expert slots per core
C = 512              # per-expert token capacity on device
P = 128
KH = H // P          # 16 k-tiles over hidden
MF = F // P          # 6  m-tiles over intermediate
KF = F // P          # 6  k-tiles over intermediate (down proj)
MH = H // P          # 16 m-tiles over hidden (down proj)

BF16 = mybir.dt.bfloat16
F32 = mybir.dt.float32

# Exposed for test harnesses: the BassKernelResults of the last device run.
LAST_RESULT = None

_NC_CACHE = None


def _build_graph():
    """One SPMD graph, identical for all 8 cores (only input data differs)."""
    nc = bacc.Bacc("TRN2", target_bir_lowering=False, debug=False,
                   num_devices=NCORES)
    xt_d = nc.dram_tensor("xt", [SLOTS, H, C], BF16, kind="ExternalInput").ap()
    wg_d = nc.dram_tensor("wg", [SLOTS, H, F], BF16, kind="ExternalInput").ap()
    wu_d = nc.dram_tensor("wu", [SLOTS, H, F], BF16, kind="ExternalInput").ap()
    wd_d = nc.dram_tensor("wd", [SLOTS, F, H], BF16, kind="ExternalInput").ap()
    y_d = nc.dram_tensor("y", [SLOTS, H, C], BF16, kind="ExternalOutput").ap()

    with tile.TileContext(nc) as tc:
        with (
            tc.tile_pool(name="xp", bufs=2 * KH) as xp,
            tc.tile_pool(name="wgp", bufs=20) as wgp,
            tc.tile_pool(name="wup", bufs=20) as wup,
            tc.tile_pool(name="wdp", bufs=2 * KF) as wdp,
            tc.tile_pool(name="hp", bufs=2 * MF) as hp,
            tc.tile_pool(name="sp", bufs=3) as sp,
            tc.tile_pool(name="yp", bufs=4) as yp,
            tc.tile_pool(name="ps", bufs=2, space="PSUM") as ps,
        ):
            def preload_gate_up(s):
                """Emit DMAs for slot s's x tiles + gate/up weights,
                interleaved so the k=0 tiles land first."""
                x_t, wg_t, wu_t = [], [], []
                for k in range(KH):
                    xt = xp.tile([P, C], BF16, tag="xt")
                    nc.sync.dma_start(
                        xt[:], xt_d[s, k * P:(k + 1) * P, :]
                    )
                    x_t.append(xt)
                    t = wgp.tile([P, F], BF16, tag="wg")
                    nc.sync.dma_start(t[:], wg_d[s, k * P:(k + 1) * P, :])
                    wg_t.append(t)
                    t = wup.tile([P, F], BF16, tag="wu")
                    nc.sync.dma_start(t[:], wu_d[s, k * P:(k + 1) * P, :])
                    wu_t.append(t)
                return x_t, wg_t, wu_t

            def preload_down(s):
                wd_t = []
                for k in range(KF):
                    t = wdp.tile([P, H], BF16, tag="wd")
                    nc.sync.dma_start(t[:], wd_d[s, k * P:(k + 1) * P, :])
                    wd_t.append(t)
                return wd_t

            nxt = preload_gate_up(0)
            for s in range(SLOTS):
                x_t, wg_t, wu_t = nxt
                wd_t = preload_down(s)

                # gate/up projections + swiglu, per F m-tile
                h_tiles = []
                for m in range(MF):
                    psg = ps.tile([P, C], F32, tag="psg")
                    for k in range(KH):
                        nc.tensor.matmul(
                            psg[:], wg_t[k][:, m * P:(m + 1) * P], x_t[k][:],
                            start=(k == 0), stop=(k == KH - 1),
                        )
                    psu = ps.tile([P, C], F32, tag="psu")
                    for k in range(KH):
                        nc.tensor.matmul(
                            psu[:], wu_t[k][:, m * P:(m + 1) * P], x_t[k][:],
                            start=(k == 0), stop=(k == KH - 1),
                        )
                    sil = sp.tile([P, C], F32, tag="sil")
                    nc.scalar.activation(
                        sil[:], psg[:], mybir.ActivationFunctionType.Silu
                    )
                    hm = hp.tile([P, C], BF16, tag="h")
                    nc.vector.tensor_tensor(
                        hm[:], sil[:], psu[:], mybir.AluOpType.mult
                    )
                    h_tiles.append(hm)

                # prefetch next slot's x + gate/up weights during down proj
                if s + 1 < SLOTS:
                    nxt = preload_gate_up(s + 1)

                # down projection
                for mh in range(MH):
                    psy = ps.tile([P, C], F32, tag="psy")
                    for k in range(KF):
                        nc.tensor.matmul(
                            psy[:], wd_t[k][:, mh * P:(mh + 1) * P], h_tiles[k][:],
                            start=(k == 0), stop=(k == KF - 1),
                        )
                    yt = yp.tile([P, C], BF16, tag="y")
                    nc.vector.tensor_copy(out=yt[:], in_=psy[:])
                    nc.sync.dma_start(y_d[s, mh * P:(mh + 1) * P, :], yt[:])

    nc.compile()
    return nc


def _route(x, gate_w):
    """Replicate the reference router. Returns (topk_idx, topk_w) as numpy."""
    try:
        import jax
        import jax.numpy as jnp

        cpu = jax.devices("cpu")[0]
        with jax.default_device(cpu):
            logits = jnp.asarray(x) @ jnp.asarray(gate_w)
            probs = jax.nn.softmax(logits.astype(jnp.float32), axis=-1)
            topk_w, topk_idx = jax.lax.top_k(probs, K)
            topk_w = topk_w / jnp.sum(topk_w, axis=-1, keepdims=True)
            return np.asarray(topk_idx), np.asarray(topk_w)
    except Exception:
        logits = x.astype(np.float32) @ gate_w.astype(np.float32)
        lm = logits.max(-1, keepdims=True)
        p = np.exp(logits - lm)
        p /= p.sum(-1, keepdims=True)
        topk_idx = np.argsort(-p, kind="stable", axis=-1)[:, :K]
        topk_w = np.take_along_axis(p, topk_idx, axis=-1)
        topk_w = topk_w / topk_w.sum(-1, keepdims=True)
        return topk_idx.astype(np.int32), topk_w


def _silu(v):
    return v / (1.0 + np.exp(-v))


def kernel(hidden_states, gate_w, w_gate_proj, w_up_proj, w_down_proj):
    global LAST_RESULT, _NC_CACHE

    x = np.asarray(hidden_states, dtype=np.float32)
    gate_w = np.asarray(gate_w, dtype=np.float32)
    wg_all = np.asarray(w_gate_proj, dtype=np.float32)
    wu_all = np.asarray(w_up_proj, dtype=np.float32)
    wd_all = np.asarray(w_down_proj, dtype=np.float32)

    # ---- Host router ----
    topk_idx, topk_w = _route(x, gate_w)

    # Per-expert token lists (kept on device up to capacity C; rest on host)
    route_w = np.zeros((T, E), np.float32)
    np.put_along_axis(route_w, topk_idx, topk_w.astype(np.float32), axis=-1)
    expert_tokens = [np.nonzero(route_w[:, e])[0] for e in range(E)]

    x_bf = x.astype(ml_dtypes.bfloat16)

    # ---- Build per-core inputs ----
    in_maps = []
    for core in range(NCORES):
        experts = [core + NCORES * s for s in range(SLOTS)]
        xt = np.zeros((SLOTS, H, C), ml_dtypes.bfloat16)
        for s, e in enumerate(experts):
            idx = expert_tokens[e][:C]
            xt[s, :, : len(idx)] = x_bf[idx].T
        in_maps.append(
            {
                "xt": xt,
                "wg": np.ascontiguousarray(wg_all[experts]).astype(ml_dtypes.bfloat16),
                "wu": np.ascontiguousarray(wu_all[experts]).astype(ml_dtypes.bfloat16),
                "wd": np.ascontiguousarray(wd_all[experts]).astype(ml_dtypes.bfloat16),
            }
        )

    # ---- Device run ----
    if _NC_CACHE is None:
        _NC_CACHE = _build_graph()
    nc = _NC_CACHE
    res = run_bass_kernel_spmd(nc, in_maps, core_ids=list(range(NCORES)))
    LAST_RESULT = res

    # ---- Host combine ----
    out = np.zeros((T, H), np.float32)
    for e in range(E):
        core, s = e % NCORES, e // NCORES
        idx = expert_tokens[e]
        kept, ov = idx[:C], idx[C:]
        yT = np.asarray(res.results[core]["y"][s]).astype(np.float32)  # [H, C]
        w_kept = route_w[kept, e]
        out[kept] += w_kept[:, None] * yT[:, : len(kept)].T
        if len(ov):
            xo = x[ov]
            h = _silu(xo @ wg_all[e]) * (xo @ wu_all[e])
            out[ov] += route_w[ov, e][:, None] * (h @ wd_all[e])

    return out


# revision 5
# speedup vs baseline: 1.0385x; 1.0385x over previous
"""Routed (sparse) MoE kernel for Trainium2, expert-parallel over 8 NeuronCores.

Problem: Qwen3-MoE sparse block. T=2048 tokens, H=2048 hidden, E=32 experts,
F=768 intermediate, top-K=8, norm_topk_prob=True.

Strategy:
  * Host: router (logits -> softmax -> top-8 -> renormalize), replicated with
    jax-on-CPU to match the reference's numerics bit-for-bit where possible.
  * Host: gather each expert's routed tokens into a fixed-capacity (512) slot,
    pre-transposed to [H, C] and cast to bf16. Expert e -> core e%8, slot e//8.
    Tokens beyond capacity (rare: mean count is 512) are computed on host in
    fp32 — this keeps the device graph shape input-independent.
  * Device (per core): 4 expert slots. For each slot, the whole SwiGLU FFN in
    a transposed dataflow (tokens on the matmul free axis), bf16 matmuls with
    fp32 PSUM accumulation, silu on ACT, multiply on DVE:
        gT[F,C] = Wg^T x      (lhsT = Wg[H,F] tiles, rhs = xT[H,C] tiles)
        uT[F,C] = Wu^T x
        hT      = silu(gT) * uT
        yT[H,C] = Wd^T h      (lhsT = Wd[F,H] tiles, rhs = hT tiles)
    No on-chip transposes anywhere.
  * Host: combine — out[t] = sum_k w[t,k] * y_{e_k}[t], a per-expert weighted
    scatter-add with unique indices (fp32).
"""

import numpy as np
import ml_dtypes

import concourse.bass as bass  # noqa: F401  (registers engines)
import concourse.mybir as mybir
import concourse.tile as tile
from concourse import bacc
from concourse.bass_utils import run_bass_kernel_spmd

# Model dims (hardcoded per problem spec)
T, H, E, F, K = 2048, 2048, 32, 768, 8
NCORES = 8
SLOTS = E // NCORES  # 4 expert slots per core
C = 512              # per-expert token capacity on device
P = 128
KH = H // P          # 16 k-tiles over hidden
MF = F // P          # 6  m-tiles over intermediate
KF = F // P          # 6  k-tiles over intermediate (down proj)
MH = H // P          # 16 m-tiles over hidden (down proj)

BF16 = mybir.dt.bfloat16
F32 = mybir.dt.float32

# Exposed for test harnesses: the BassKernelResults of the last device run.
LAST_RESULT = None

_NC_CACHE = None


def _build_graph():
    """One SPMD graph, identical for all 8 cores (only input data differs)."""
    nc = bacc.Bacc("TRN2", target_bir_lowering=False, debug=False,
                   num_devices=NCORES)
    xt_d = nc.dram_tensor("xt", [SLOTS, H, C], BF16, kind="ExternalInput").ap()
    wg_d = nc.dram_tensor("wg", [SLOTS, H, F], BF16, kind="ExternalInput").ap()
    wu_d = nc.dram_tensor("wu", [SLOTS, H, F], BF16, kind="ExternalInput").ap()
    wd_d = nc.dram_tensor("wd", [SLOTS, F, H], BF16, kind="ExternalInput").ap()
    y_d = nc.dram_tensor("y", [SLOTS, H, C], BF16, kind="ExternalOutput").ap()

    with tile.TileContext(nc) as tc:
        with (
            tc.tile_pool(name="xp", bufs=2) as xp,
            tc.tile_pool(name="wgp", bufs=20) as wgp,
            tc.tile_pool(name="wup", bufs=20) as wup,
            tc.tile_pool(name="wdp", bufs=2 * KF) as wdp,
            tc.tile_pool(name="hp", bufs=2 * MF) as hp,
            tc.tile_pool(name="sp", bufs=3) as sp,
            tc.tile_pool(name="yp", bufs=4) as yp,
            tc.tile_pool(name="ps", bufs=2, space="PSUM") as ps,
        ):
            def preload_gate_up(s):
                """Emit DMAs for slot s's x tile + gate/up weights."""
                xt = xp.tile([P, KH, C], BF16, tag="xt")
                nc.sync.dma_start(
                    xt[:], xt_d[s].rearrange("(ko p) c -> p ko c", p=P)
                )
                wg_t, wu_t = [], []
                for k in range(KH):
                    t = wgp.tile([P, F], BF16, tag="wg")
                    nc.sync.dma_start(t[:], wg_d[s, k * P:(k + 1) * P, :])
                    wg_t.append(t)
                    t = wup.tile([P, F], BF16, tag="wu")
                    nc.sync.dma_start(t[:], wu_d[s, k * P:(k + 1) * P, :])
                    wu_t.append(t)
                return xt, wg_t, wu_t

            def preload_down(s):
                wd_t = []
                for k in range(KF):
                    t = wdp.tile([P, H], BF16, tag="wd")
                    nc.sync.dma_start(t[:], wd_d[s, k * P:(k + 1) * P, :])
                    wd_t.append(t)
                return wd_t

            nxt = preload_gate_up(0)
            for s in range(SLOTS):
                xt, wg_t, wu_t = nxt
                wd_t = preload_down(s)

                # gate/up projections + swiglu, per F m-tile
                h_tiles = []
                for m in range(MF):
                    psg = ps.tile([P, C], F32, tag="psg")
                    for k in range(KH):
                        nc.tensor.matmul(
                            psg[:], wg_t[k][:, m * P:(m + 1) * P], xt[:, k, :],
                            start=(k == 0), stop=(k == KH - 1),
                        )
                    psu = ps.tile([P, C], F32, tag="psu")
                    for k in range(KH):
                        nc.tensor.matmul(
                            psu[:], wu_t[k][:, m * P:(m + 1) * P], xt[:, k, :],
                            start=(k == 0), stop=(k == KH - 1),
                        )
                    sil = sp.tile([P, C], F32, tag="sil")
                    nc.scalar.activation(
                        sil[:], psg[:], mybir.ActivationFunctionType.Silu
                    )
                    hm = hp.tile([P, C], BF16, tag="h")
                    nc.vector.tensor_tensor(
                        hm[:], sil[:], psu[:], mybir.AluOpType.mult
                    )
                    h_tiles.append(hm)

                # prefetch next slot's x + gate/up weights during down proj
                if s + 1 < SLOTS:
                    nxt = preload_gate_up(s + 1)

                # down projection
                for mh in range(MH):
                    psy = ps.tile([P, C], F32, tag="psy")
                    for k in range(KF):
                        nc.tensor.matmul(
                            psy[:], wd_t[k][:, mh * P:(mh + 1) * P], h_tiles[k][:],
                            start=(k == 0), stop=(k == KF - 1),
                        )
                    yt = yp.tile([P, C], BF16, tag="y")
                    nc.vector.tensor_copy(out=yt[:], in_=psy[:])
                    nc.sync.dma_start(y_d[s, mh * P:(mh + 1) * P, :], yt[:])

    nc.compile()
    return nc


def _route(x, gate_w):
    """Replicate the reference router. Returns (topk_idx, topk_w) as numpy."""
    try:
        import jax
        import jax.numpy as jnp

        cpu = jax.devices("cpu")[0]
        with jax.default_device(cpu):
            logits = jnp.asarray(x) @ jnp.asarray(gate_w)
            probs = jax.nn.softmax(logits.astype(jnp.float32), axis=-1)
            topk_w, topk_idx = jax.lax.top_k(probs, K)
            topk_w = topk_w / jnp.sum(topk_w, axis=-1, keepdims=True)
            return np.asarray(topk_idx), np.asarray(topk_w)
    except Exception:
        logits = x.astype(np.float32) @ gate_w.astype(np.float32)
        lm = logits.max(-1, keepdims=True)
        p = np.exp(logits - lm)
        p /= p.sum(-1, keepdims=True)
        topk_idx = np.argsort(-p, kind="stable", axis=-1)[:, :K]
        topk_w = np.take_along_axis(p, topk_idx, axis=-1)
        topk_w = topk_w / topk_w.sum(-1, keepdims=True)
        return topk_idx.astype(np.int32), topk_w


def _silu(v):
    return v / (1.0 + np.exp(-v))


def kernel(hidden_states, gate_w, w_gate_proj, w_up_proj, w_down_proj):
    global LAST_RESULT, _NC_CACHE

    x = np.asarray(hidden_states, dtype=np.float32)
    gate_w = np.asarray(gate_w, dtype=np.float32)
    wg_all = np.asarray(w_gate_proj, dtype=np.float32)
    wu_all = np.asarray(w_up_proj, dtype=np.float32)
    wd_all = np.asarray(w_down_proj, dtype=np.float32)

    # ---- Host router ----
    topk_idx, topk_w = _route(x, gate_w)

    # Per-expert token lists (kept on device up to capacity C; rest on host)
    route_w = np.zeros((T, E), np.float32)
    np.put_along_axis(route_w, topk_idx, topk_w.astype(np.float32), axis=-1)
    expert_tokens = [np.nonzero(route_w[:, e])[0] for e in range(E)]

    x_bf = x.astype(ml_dtypes.bfloat16)

    # ---- Build per-core inputs ----
    in_maps = []
    for core in range(NCORES):
        experts = [core + NCORES * s for s in range(SLOTS)]
        xt = np.zeros((SLOTS, H, C), ml_dtypes.bfloat16)
        for s, e in enumerate(experts):
            idx = expert_tokens[e][:C]
            xt[s, :, : len(idx)] = x_bf[idx].T
        in_maps.append(
            {
                "xt": xt,
                "wg": np.ascontiguousarray(wg_all[experts]).astype(ml_dtypes.bfloat16),
                "wu": np.ascontiguousarray(wu_all[experts]).astype(ml_dtypes.bfloat16),
                "wd": np.ascontiguousarray(wd_all[experts]).astype(ml_dtypes.bfloat16),
            }
        )

    # ---- Device run ----
    if _NC_CACHE is None:
        _NC_CACHE = _build_graph()
    nc = _NC_CACHE
    res = run_bass_kernel_spmd(nc, in_maps, core_ids=list(range(NCORES)))
    LAST_RESULT = res

    # ---- Host combine ----
    out = np.zeros((T, H), np.float32)
    for e in range(E):
        core, s = e % NCORES, e // NCORES
        idx = expert_tokens[e]
        kept, ov = idx[:C], idx[C:]
        yT = np.asarray(res.results[core]["y"][s]).astype(np.float32)  # [H, C]
        w_kept = route_w[kept, e]
        out[kept] += w_kept[:, None] * yT[:, : len(kept)].T
        if len(ov):
            xo = x[ov]
            h = _silu(xo @ wg_all[e]) * (xo @ wu_all[e])
            out[ov] += route_w[ov, e][:, None] * (h @ wd_all[e])

    return out


# revision 6
# speedup vs baseline: 1.0696x; 1.0299x over previous
"""Routed (sparse) MoE kernel for Trainium2, expert-parallel over 8 NeuronCores.

Problem: Qwen3-MoE sparse block. T=2048 tokens, H=2048 hidden, E=32 experts,
F=768 intermediate, top-K=8, norm_topk_prob=True.

Strategy:
  * Host: router (logits -> softmax -> top-8 -> renormalize), replicated with
    jax-on-CPU to match the reference's numerics bit-for-bit where possible.
  * Host: gather each expert's routed tokens into a fixed-capacity (512) slot,
    pre-transposed to [H, C] and cast to bf16. Expert e -> core e%8, slot e//8.
    Tokens beyond capacity (rare: mean count is 512) are computed on host in
    fp32 — this keeps the device graph shape input-independent.
  * Device (per core): 4 expert slots. For each slot, the whole SwiGLU FFN in
    a transposed dataflow (tokens on the matmul free axis), bf16 matmuls with
    fp32 PSUM accumulation, silu on ACT, multiply on DVE:
        gT[F,C] = Wg^T x      (lhsT = Wg[H,F] tiles, rhs = xT[H,C] tiles)
        uT[F,C] = Wu^T x
        hT      = silu(gT) * uT
        yT[H,C] = Wd^T h      (lhsT = Wd[F,H] tiles, rhs = hT tiles)
    No on-chip transposes anywhere.
  * Host: combine — out[t] = sum_k w[t,k] * y_{e_k}[t], a per-expert weighted
    scatter-add with unique indices (fp32).
"""

import numpy as np
import ml_dtypes

import concourse.bass as bass  # noqa: F401  (registers engines)
import concourse.mybir as mybir
import concourse.tile as tile
from concourse import bacc
from concourse.bass_utils import run_bass_kernel_spmd

# Model dims (hardcoded per problem spec)
T, H, E, F, K = 2048, 2048, 32, 768, 8
NCORES = 8
SLOTS = E // NCORES  # 4 expert slots per core
C = 512              # per-expert token capacity on device
P = 128
KH = H // P          # 16 k-tiles over hidden
MF = F // P          # 6  m-tiles over intermediate
KF = F // P          # 6  k-tiles over intermediate (down proj)
MH = H // P          # 16 m-tiles over hidden (down proj)

BF16 = mybir.dt.bfloat16
F32 = mybir.dt.float32

# Exposed for test harnesses: the BassKernelResults of the last device run.
LAST_RESULT = None

_NC_CACHE = None


def _build_graph():
    """One SPMD graph, identical for all 8 cores (only input data differs)."""
    nc = bacc.Bacc("TRN2", target_bir_lowering=False, debug=False,
                   num_devices=NCORES)
    xt_d = nc.dram_tensor("xt", [SLOTS, H, C], BF16, kind="ExternalInput").ap()
    wg_d = nc.dram_tensor("wg", [SLOTS, H, F], BF16, kind="ExternalInput").ap()
    wu_d = nc.dram_tensor("wu", [SLOTS, H, F], BF16, kind="ExternalInput").ap()
    wd_d = nc.dram_tensor("wd", [SLOTS, F, H], BF16, kind="ExternalInput").ap()
    y_d = nc.dram_tensor("y", [SLOTS, H, C], BF16, kind="ExternalOutput").ap()

    with tile.TileContext(nc) as tc:
        with (
            tc.tile_pool(name="xp", bufs=8) as xp,
            tc.tile_pool(name="wgp", bufs=32) as wgp,
            tc.tile_pool(name="wup", bufs=20) as wup,
            tc.tile_pool(name="wdp", bufs=8) as wdp,
            tc.tile_pool(name="hp", bufs=2 * MF) as hp,
            tc.tile_pool(name="sp", bufs=3) as sp,
            tc.tile_pool(name="yp", bufs=8) as yp,
            tc.tile_pool(name="ps", bufs=2, space="PSUM") as ps,
        ):
            XCH = 4  # k-tiles per x chunk

            def preload_gate_up(s):
                """Emit DMAs for slot s's x chunks + gate/up weights,
                ordered so chunk 0 + early wg tiles land first."""
                x_t, wg_t, wu_t = [], [], []
                for c in range(KH // XCH):
                    xc = xp.tile([P, XCH, C], BF16, tag="xt")
                    nc.sync.dma_start(
                        xc[:],
                        xt_d[s, c * XCH * P:(c + 1) * XCH * P, :].rearrange(
                            "(ko p) c -> p ko c", p=P
                        ),
                    )
                    x_t.append(xc)
                    for k in range(c * XCH, (c + 1) * XCH):
                        t = wgp.tile([P, F], BF16, tag="wg")
                        nc.sync.dma_start(t[:], wg_d[s, k * P:(k + 1) * P, :])
                        wg_t.append(t)
                for k in range(KH):
                    t = wup.tile([P, F], BF16, tag="wu")
                    nc.sync.dma_start(t[:], wu_d[s, k * P:(k + 1) * P, :])
                    wu_t.append(t)
                return x_t, wg_t, wu_t

            def preload_down(s):
                wd_t = []
                for k in range(KF):
                    t = wdp.tile([P, H], BF16, tag="wd")
                    nc.sync.dma_start(t[:], wd_d[s, k * P:(k + 1) * P, :])
                    wd_t.append(t)
                return wd_t

            nxt = preload_gate_up(0)
            for s in range(SLOTS):
                x_t, wg_t, wu_t = nxt
                wd_t = preload_down(s)

                # gate/up projections + swiglu, per F m-tile
                h_tiles = []
                for m in range(MF):
                    psg = ps.tile([P, C], F32, tag="psg")
                    for k in range(KH):
                        nc.tensor.matmul(
                            psg[:], wg_t[k][:, m * P:(m + 1) * P],
                            x_t[k // XCH][:, k % XCH, :],
                            start=(k == 0), stop=(k == KH - 1),
                        )
                    psu = ps.tile([P, C], F32, tag="psu")
                    for k in range(KH):
                        nc.tensor.matmul(
                            psu[:], wu_t[k][:, m * P:(m + 1) * P],
                            x_t[k // XCH][:, k % XCH, :],
                            start=(k == 0), stop=(k == KH - 1),
                        )
                    sil = sp.tile([P, C], F32, tag="sil")
                    nc.scalar.activation(
                        sil[:], psg[:], mybir.ActivationFunctionType.Silu
                    )
                    hm = hp.tile([P, C], BF16, tag="h")
                    nc.vector.tensor_tensor(
                        hm[:], sil[:], psu[:], mybir.AluOpType.mult
                    )
                    h_tiles.append(hm)
                    # prefetch next slot's x + gate/up weights early
                    if m == 1 and s + 1 < SLOTS:
                        nxt = preload_gate_up(s + 1)

                # down projection
                for mh in range(MH):
                    psy = ps.tile([P, C], F32, tag="psy")
                    for k in range(KF):
                        nc.tensor.matmul(
                            psy[:], wd_t[k][:, mh * P:(mh + 1) * P], h_tiles[k][:],
                            start=(k == 0), stop=(k == KF - 1),
                        )
                    yt = yp.tile([P, C], BF16, tag="y")
                    nc.vector.tensor_copy(out=yt[:], in_=psy[:])
                    nc.gpsimd.dma_start(y_d[s, mh * P:(mh + 1) * P, :], yt[:])

    nc.compile()
    return nc


def _route(x, gate_w):
    """Replicate the reference router. Returns (topk_idx, topk_w) as numpy."""
    try:
        import jax
        import jax.numpy as jnp

        cpu = jax.devices("cpu")[0]
        with jax.default_device(cpu):
            logits = jnp.asarray(x) @ jnp.asarray(gate_w)
            probs = jax.nn.softmax(logits.astype(jnp.float32), axis=-1)
            topk_w, topk_idx = jax.lax.top_k(probs, K)
            topk_w = topk_w / jnp.sum(topk_w, axis=-1, keepdims=True)
            return np.asarray(topk_idx), np.asarray(topk_w)
    except Exception:
        logits = x.astype(np.float32) @ gate_w.astype(np.float32)
        lm = logits.max(-1, keepdims=True)
        p = np.exp(logits - lm)
        p /= p.sum(-1, keepdims=True)
        topk_idx = np.argsort(-p, kind="stable", axis=-1)[:, :K]
        topk_w = np.take_along_axis(p, topk_idx, axis=-1)
        topk_w = topk_w / topk_w.sum(-1, keepdims=True)
        return topk_idx.astype(np.int32), topk_w


def _silu(v):
    return v / (1.0 + np.exp(-v))


def kernel(hidden_states, gate_w, w_gate_proj, w_up_proj, w_down_proj):
    global LAST_RESULT, _NC_CACHE

    x = np.asarray(hidden_states, dtype=np.float32)
    gate_w = np.asarray(gate_w, dtype=np.float32)
    wg_all = np.asarray(w_gate_proj, dtype=np.float32)
    wu_all = np.asarray(w_up_proj, dtype=np.float32)
    wd_all = np.asarray(w_down_proj, dtype=np.float32)

    # ---- Host router ----
    topk_idx, topk_w = _route(x, gate_w)

    # Per-expert token lists (kept on device up to capacity C; rest on host)
    route_w = np.zeros((T, E), np.float32)
    np.put_along_axis(route_w, topk_idx, topk_w.astype(np.float32), axis=-1)
    expert_tokens = [np.nonzero(route_w[:, e])[0] for e in range(E)]

    x_bf = x.astype(ml_dtypes.bfloat16)

    # ---- Build per-core inputs ----
    in_maps = []
    for core in range(NCORES):
        experts = [core + NCORES * s for s in range(SLOTS)]
        xt = np.zeros((SLOTS, H, C), ml_dtypes.bfloat16)
        for s, e in enumerate(experts):
            idx = expert_tokens[e][:C]
            xt[s, :, : len(idx)] = x_bf[idx].T
        in_maps.append(
            {
                "xt": xt,
                "wg": np.ascontiguousarray(wg_all[experts]).astype(ml_dtypes.bfloat16),
                "wu": np.ascontiguousarray(wu_all[experts]).astype(ml_dtypes.bfloat16),
                "wd": np.ascontiguousarray(wd_all[experts]).astype(ml_dtypes.bfloat16),
            }
        )

    # ---- Device run ----
    if _NC_CACHE is None:
        _NC_CACHE = _build_graph()
    nc = _NC_CACHE
    res = run_bass_kernel_spmd(nc, in_maps, core_ids=list(range(NCORES)))
    LAST_RESULT = res

    # ---- Host combine ----
    out = np.zeros((T, H), np.float32)
    for e in range(E):
        core, s = e % NCORES, e // NCORES
        idx = expert_tokens[e]
        kept, ov = idx[:C], idx[C:]
        yT = np.asarray(res.results[core]["y"][s]).astype(np.float32)  # [H, C]
        w_kept = route_w[kept, e]
        out[kept] += w_kept[:, None] * yT[:, : len(kept)].T
        if len(ov):
            xo = x[ov]
            h = _silu(xo @ wg_all[e]) * (xo @ wu_all[e])
            out[ov] += route_w[ov, e][:, None] * (h @ wd_all[e])

    return out
